# revision 2
# baseline (speedup 1.0000x reference)
"""Attention-decoder (B=128, T=256, F=512, O=512, MID=1000, 32 steps) on 8 trn2 cores.

v3: everything transposed. All per-step matmuls produce [128, tiny] outputs
(out-free-size is what the PE pays for): logits come out as logitsT [t, b]
columns (lhsT = tanh tile, rhs = W2 column), context as ctxT [f, b] (lhsT =
aN chunk, rhs = e column), gates as gT [o, b] (lhsT = WgT chunk, rhs = xaT).
The LSTM state stays in [o-chunk, b] layout so no transposes exist anywhere.
Softmax is unnormalized exp (exp(relu(x)) == max(exp(x),1), logits < 5) with
the 1/sum folded into the e-scatter; sums via ones-column matmuls.
"""
import sys
import numpy as np

sys.path.insert(0, "/opt/trn_rl_repo")

B, T, F, O, MID = 128, 256, 512, 512, 1000
MIDP = 1024  # padded
NCORES = 8
BC = B // NCORES  # 16 batch per core
BT = BC * T       # 4096


def _build(wo: int):
    import concourse.bass as bass
    import concourse.bacc as bacc
    import concourse.mybir as mybir
    from concourse.tile import TileContext

    f16 = mybir.dt.float16
    f32 = mybir.dt.float32
    AF = mybir.ActivationFunctionType
    OP = mybir.AluOpType

    nc = bacc.Bacc()
    aT_d = nc.dram_tensor("aT", [F, BT], f16, kind="ExternalInput")
    aN_d = nc.dram_tensor("aN", [BT, F], f16, kind="ExternalInput")
    W1aT_d = nc.dram_tensor("W1aT", [F, MIDP], f16, kind="ExternalInput")
    W1sT_d = nc.dram_tensor("W1sT", [O, MIDP], f16, kind="ExternalInput")
    W2c_d = nc.dram_tensor("W2c", [128, 8], f16, kind="ExternalInput")
    b1T_d = nc.dram_tensor("b1T", [128, 8], f32, kind="ExternalInput")
    b2c_d = nc.dram_tensor("b2c", [128, 1], f32, kind="ExternalInput")
    WgT_d = nc.dram_tensor("WgT", [O + F, 4 * O], f16, kind="ExternalInput")
    bgT2_d = nc.dram_tensor("bgT2", [16, 128], f16, kind="ExternalInput")
    ind_d = nc.dram_tensor("ind16", [16, 256], f16, kind="ExternalInput")
    zrow_d = nc.dram_tensor("zrow", [1, 64], f16, kind="ExternalInput")
    onesrh_d = nc.dram_tensor("onesrh", [1, 128], f16, kind="ExternalInput")
    sPT_d = nc.dram_tensor("sPT", [128, 64], f16, kind="ExternalInput")
    onesc_d = nc.dram_tensor("onesc", [128, 1], f16, kind="ExternalInput")
    onesr_d = nc.dram_tensor("onesr", [1, 128], f32, kind="ExternalInput")
    ones_d = nc.dram_tensor("ones16", [1, BC], f16, kind="ExternalInput")
    out_d = nc.dram_tensor("out", [wo, 128, 64], f32, kind="ExternalOutput")

    with TileContext(nc) as tc:
        with (
            tc.tile_pool(name="const", bufs=1) as cp,
            tc.tile_pool(name="hq", bufs=3) as hqp,
            tc.tile_pool(name="th", bufs=3) as thp,
            tc.tile_pool(name="wp", bufs=2) as wp,
            tc.tile_pool(name="small", bufs=2) as sp,
            tc.tile_pool(name="astream", bufs=1) as app,
            tc.tile_pool(name="pstf", bufs=2, space="PSUM") as pst,
            tc.tile_pool(name="psbig", bufs=1, space="PSUM") as psbig,
            tc.tile_pool(name="pslt", bufs=2, space="PSUM") as plp,
            tc.tile_pool(name="psct", bufs=1, space="PSUM") as pcp,
            tc.tile_pool(name="psgt", bufs=1, space="PSUM") as pgp,
        ):
            dma = nc.sync.dma_start

            # ---- constant loads ----
            aN_sb = {}
            for b in range(BC):
                for tcn in range(2):
                    t_ = cp.tile([128, F], f16, tag=f"aN{b}_{tcn}", name=f"aN{b}_{tcn}")
                    dma(t_[:], aN_d[b * T + tcn * 128: b * T + (tcn + 1) * 128, :])
                    aN_sb[(b, tcn)] = t_
            W1aT_sb, W1sT_sb, WgT_sb = [], [], []
            for kc in range(4):
                t_ = cp.tile([128, MIDP], f16, tag=f"w1a{kc}", name=f"w1a{kc}")
                dma(t_[:], W1aT_d[kc * 128:(kc + 1) * 128, :])
                W1aT_sb.append(t_)
            for kc in range(4):
                t_ = cp.tile([128, MIDP], f16, tag=f"w1s{kc}", name=f"w1s{kc}")
                dma(t_[:], W1sT_d[kc * 128:(kc + 1) * 128, :])
                W1sT_sb.append(t_)
            for kc in range(8):
                t_ = cp.tile([128, 4 * O], f16, tag=f"wg{kc}", name=f"wg{kc}")
                dma(t_[:], WgT_d[kc * 128:(kc + 1) * 128, :])
                WgT_sb.append(t_)
            W2_sb = cp.tile([128, 8], f16, tag="w2", name="w2")
            dma(W2_sb[:], W2c_d[:])
            b1T_sb = cp.tile([128, 8], f32, tag="b1t", name="b1t")
            dma(b1T_sb[:], b1T_d[:])
            b2c_sb = cp.tile([128, 1], f32, tag="b2c", name="b2c")
            dma(b2c_sb[:], b2c_d[:])
            bgT2_sb = cp.tile([16, 128], f16, tag="bgT2", name="bgT2")
            dma(bgT2_sb[:], bgT2_d[:])
            ind_sb = cp.tile([16, 256], f16, tag="ind16", name="ind16")
            dma(ind_sb[:], ind_d[:])
            zrow_sb = cp.tile([1, 64], f16, tag="zrow", name="zrow")
            dma(zrow_sb[:], zrow_d[:])
            onesrh_sb = cp.tile([1, 128], f16, tag="onesrh", name="onesrh")
            dma(onesrh_sb[:], onesrh_d[:])
            onesc_sb = cp.tile([128, 1], f16, tag="onesc", name="onesc")
            dma(onesc_sb[:], onesc_d[:])
            onesr_sb = cp.tile([1, 128], f32, tag="onesr", name="onesr")
            dma(onesr_sb[:], onesr_d[:])
            ones_sb = cp.tile([1, BC], f16, tag="ones", name="ones")
            dma(ones_sb[:], ones_d[:])

            s16 = wp.tile([128, 64], f16, tag="s16", name="s16")
            dma(s16[:], sPT_d[:])
            cT_prev = wp.tile([128, 64], f32, tag="cT", name="cT")
            nc.vector.memset(cT_prev[:], 0.0)

            # ---- precompute pre = (a @ W1a.T).T : [MID_p, (b,t)] fp16 ----
            pre_sb = []
            for mc in range(8):
                pre_sb.append(cp.tile([128, BT], f16, tag=f"pre{mc}", name=f"pre{mc}"))
            for ns in range(8):
                a_sl = []
                for kc in range(4):
                    t_ = app.tile([128, 512], f16, tag=f"astr{kc}", name=f"astr{kc}")
                    dma(t_[:], aT_d[kc * 128:(kc + 1) * 128, ns * 512:(ns + 1) * 512])
                    a_sl.append(t_)
                for mc in range(8):
                    ps = psbig.tile([128, 512], f32, tag="psbig", name="psbig")
                    for kc in range(4):
                        nc.tensor.matmul(
                            ps[:],
                            W1aT_sb[kc][:, mc * 128:(mc + 1) * 128],
                            a_sl[kc][:],
                            start=(kc == 0), stop=(kc == 3),
                        )
                    dst = pre_sb[mc][:, ns * 512:(ns + 1) * 512]
                    if mc % 2 == 0:
                        nc.scalar.copy(dst, ps[:])
                    else:
                        nc.vector.tensor_copy(dst, ps[:])

            # ---- decode steps ----
            for t in range(wo):
                # u.T = W1s @ s.T + b1 : [MID_p(128x8), b] fp32
                uT = wp.tile([128, 128], f32, tag="uT", name="uT")
                for mc in range(8):
                    psu = pst.tile([128, BC], f32, tag="pstf", name="psu")
                    for kc in range(4):
                        nc.tensor.matmul(
                            psu[:],
                            W1sT_sb[kc][:, mc * 128:(mc + 1) * 128],
                            s16[:, kc * 16:(kc + 1) * 16],
                            start=(kc == 0), stop=(kc == 3),
                        )
                    nc.vector.tensor_scalar(
                        out=uT[:, mc * BC:(mc + 1) * BC], in0=psu[:],
                        scalar1=b1T_sb[:, mc:mc + 1], scalar2=None, op0=OP.add,
                    )

                # gates gT [128o, (g,oc,b)]: bias + s-part early; ctx in tail
                gT = pgp.tile([128, 256], f32, tag="gT", name="gT")
                nc.tensor.matmul(gT[:], bgT2_sb[:], ind_sb[:], start=True, stop=False)
                for g in range(4):
                    for oc in range(4):
                        col = (g * 4 + oc) * BC
                        for fc in range(4):
                            nc.tensor.matmul(
                                gT[:, col:col + BC],
                                WgT_sb[fc][:, g * O + oc * 128: g * O + (oc + 1) * 128],
                                s16[:, fc * 16:(fc + 1) * 16],
                                start=False, stop=False)

                # attention: logitsT columns [t, (tcn,b)] via lhsT=tanh chunks
                plqs = [None] * 4
                psS = pst.tile([1, BC], f32, tag="psS", name="psS", bufs=1)
                pcT = pcp.tile([128, 64], f32, tag="pcT", name="pcT")
                nc.tensor.matmul(pcT[:], onesrh_sb[:], zrow_sb[:], start=True, stop=False)
                einv = sp.tile([1, BC], f32, tag="einv", name="einv")
                eT = [sp.tile([128, BC], f16, tag=f"eT{i}", name=f"eT{i}", bufs=1)
                      for i in range(2)]
                invF = sp.tile([128, BC], f32, tag="invF", name="invF", bufs=1)

                def attn_tail(q):
                    # exp of quad q's logitsT columns (reads PSUM directly)
                    ET = sp.tile([128, 8], f16, tag="ET", name="ET")
                    for tcn in range(2):
                        nc.scalar.activation(
                            ET[:, tcn * 4:(tcn + 1) * 4],
                            plqs[q][:, tcn * 4:(tcn + 1) * 4],
                            AF.Exp, bias=b2c_sb[:, 0:1], scale=1.0)
                    # e = max(exp, 1)  (== exp(relu(logit)))
                    st2 = sp.tile([128, 8], f16, tag="st2", name="st2")
                    nc.vector.tensor_scalar(out=st2[:], in0=ET[:], scalar1=1.0,
                                            scalar2=None, op0=OP.max)
                    # per-batch sums via ones-column matmul (accum over tcn)
                    for tcn in range(2):
                        nc.tensor.matmul(psS[0:1, q * 4:(q + 1) * 4], onesc_sb[:],
                                         st2[:, tcn * 4:(tcn + 1) * 4],
                                         start=(tcn == 0), stop=(tcn == 1))
                    nc.vector.reciprocal(einv[0:1, q * 4:(q + 1) * 4],
                                         psS[0:1, q * 4:(q + 1) * 4])
                    # broadcast 1/S down all partitions, then scatter e*inv
                    pin = pst.tile([128, BC], f32, tag="pstf", name="pin")
                    nc.tensor.matmul(pin[:, 0:4], onesr_sb[:],
                                     einv[0:1, q * 4:(q + 1) * 4],
                                     start=True, stop=True)
                    nc.vector.tensor_copy(invF[:, q * 4:(q + 1) * 4], pin[:, 0:4])
                    for tcn in range(2):
                        for bl in range(4):
                            b = q * 4 + bl
                            nc.vector.tensor_scalar(
                                out=eT[tcn][:, b:b + 1],
                                in0=st2[:, tcn * 4 + bl:tcn * 4 + bl + 1],
                                scalar1=invF[:, b:b + 1], scalar2=None, op0=OP.mult)
                    # ctxT columns: lhsT = aN chunk, rhs = e column
                    for bl in range(4):
                        b = q * 4 + bl
                        for fc in range(4):
                            for tcn in range(2):
                                nc.tensor.matmul(
                                    pcT[:, fc * BC + b: fc * BC + b + 1],
                                    aN_sb[(b, tcn)][:, fc * 128:(fc + 1) * 128],
                                    eT[tcn][:, b:b + 1],
                                    start=False,
                                    stop=(q == 3 and bl == 3 and fc == 3 and tcn == 1))

                for q in range(4):
                    plq = plp.tile([128, 8], f32, tag="plq", name="plq")
                    plqs[q] = plq
                    nc.tensor.matmul(plq[:], onesrh_sb[:], zrow_sb[0:1, 0:8],
                                     start=True, stop=False)
                    for mcp in range(4):
                        hq_t = hqp.tile([128, 2048], f16, tag="hq", name="hq")
                        for mcl in range(2):
                            mc = mcp * 2 + mcl
                            for bl in range(4):
                                b = q * 4 + bl
                                nc.vector.tensor_scalar(
                                    out=hq_t[:, mcl * 1024 + bl * 256: mcl * 1024 + (bl + 1) * 256],
                                    in0=pre_sb[mc][:, b * 256:(b + 1) * 256],
                                    scalar1=uT[:, mc * BC + b: mc * BC + b + 1],
                                    scalar2=None, op0=OP.add,
                                )
                        th_t = thp.tile([128, 2048], f16, tag="th", name="th")
                        nc.scalar.activation(th_t[:], hq_t[:], AF.Tanh)
                        for mcl in range(2):
                            mc = mcp * 2 + mcl
                            for bl in range(4):
                                b = q * 4 + bl
                                for tcn in range(2):
                                    nc.tensor.matmul(
                                        plq[:, tcn * 4 + bl: tcn * 4 + bl + 1],
                                        th_t[:, mcl * 1024 + bl * 256 + tcn * 128:
                                             mcl * 1024 + bl * 256 + (tcn + 1) * 128],
                                        W2_sb[:, mc:mc + 1],
                                        start=False,
                                        stop=(mc == 7 and bl == 3 and tcn == 1),
                                    )
                    if q >= 1:
                        attn_tail(q - 1)
                attn_tail(3)

                # ctx -> SBUF f16 (already normalized), then gate ctx-part
                cxT = sp.tile([128, 64], f16, tag="cxT", name="cxT", bufs=1)
                nc.vector.tensor_copy(cxT[:], pcT[:])
                for g in range(4):
                    for oc in range(4):
                        col = (g * 4 + oc) * BC
                        for fc in range(4):
                            nc.tensor.matmul(
                                gT[:, col:col + BC],
                                WgT_sb[4 + fc][:, g * O + oc * 128: g * O + (oc + 1) * 128],
                                cxT[:, fc * 16:(fc + 1) * 16],
                                start=False,
                                stop=(g == 3 and oc == 3 and fc == 3))

                # activations in [128, (g,oc,b)] layout: g0=cand tanh, g1..3 sigmoid
                gact = sp.tile([128, 64], f32, tag="gact", name="gact", bufs=1)
                nc.scalar.activation(gact[:], gT[:, 0:64], AF.Tanh)
                gsg = sp.tile([128, 192], f32, tag="gsg", name="gsg", bufs=1)
                nc.scalar.activation(gsg[:], gT[:, 64:256], AF.Tanh, scale=0.5)
                gsig = sp.tile([128, 192], f32, tag="gsig", name="gsig", bufs=1)
                nc.vector.tensor_scalar(out=gsig[:], in0=gsg[:], scalar1=0.5,
                                        scalar2=0.5, op0=OP.mult, op1=OP.add)

                t1 = sp.tile([128, 64], f32, tag="t1", name="t1", bufs=1)
                nc.vector.tensor_tensor(out=t1[:], in0=gsig[:, 0:64], in1=gact[:], op=OP.mult)
                t2 = sp.tile([128, 64], f32, tag="t2", name="t2", bufs=1)
                nc.vector.tensor_tensor(out=t2[:], in0=gsig[:, 64:128], in1=cT_prev[:], op=OP.mult)
                cT_new = wp.tile([128, 64], f32, tag="cT", name="cT")
                nc.vector.tensor_tensor(out=cT_new[:], in0=t1[:], in1=t2[:], op=OP.add)
                tch = sp.tile([128, 64], f32, tag="tch", name="tch", bufs=1)
                nc.scalar.activation(tch[:], cT_new[:], AF.Tanh)
                sT32 = wp.tile([128, 64], f32, tag="sT32", name="sT32")
                nc.vector.tensor_tensor(out=sT32[:], in0=gsig[:, 128:192], in1=tch[:],
                                        op=OP.mult)

                dma(out_d[t, :, :], sT32[:])

                cT_prev = cT_new
                if t + 1 < wo:
                    s16_new = wp.tile([128, 64], f16, tag="s16", name="s16")
                    nc.vector.tensor_copy(s16_new[:], sT32[:])
                    s16 = s16_new
    nc.compile()
    return nc


def _make_runner(nc):
    """Build the sharded jit callable ONCE per module (run_bass_via_pjrt
    rebuilds it per call, costing seconds of retrace/recompile)."""
    import jax
    import numpy as _np
    from jax.sharding import Mesh, PartitionSpec
    from jax.experimental.shard_map import shard_map
    from concourse import bass2jax, mybir

    bass2jax.install_neuronx_cc_hook()
    partition_name = nc.partition_id_tensor.name if nc.partition_id_tensor else None
    in_names, out_names, out_avals, zero_outs = [], [], [], []
    for alloc in nc.m.functions[0].allocations:
        if not isinstance(alloc, mybir.MemoryLocationSet):
            continue
        name = alloc.memorylocations[0].name
        if alloc.kind == "ExternalInput":
            if name != partition_name:
                in_names.append(name)
        elif alloc.kind == "ExternalOutput":
            shape = tuple(alloc.tensor_shape)
            dtype = mybir.dt.np(alloc.dtype)
            out_names.append(name)
            out_avals.append(jax.core.ShapedArray(shape, dtype))
            zero_outs.append(_np.zeros(shape, dtype))
    n_params = len(in_names)
    n_outs = len(out_avals)
    in_names_all = list(in_names) + list(out_names)
    if partition_name is not None:
        in_names_all.append(partition_name)

    def _body(*args):
        operands = list(args)
        if partition_name is not None:
            operands.append(bass2jax.partition_id_tensor())
        outs = bass2jax._bass_exec_p.bind(
            *operands,
            out_avals=tuple(out_avals),
            in_names=tuple(in_names_all),
            out_names=tuple(out_names),
            lowering_input_output_aliases=(),
            sim_require_finite=True,
            sim_require_nnan=True,
            nc=nc,
        )
        return tuple(outs)

    donate = tuple(range(n_params, n_params + n_outs))
    devices = jax.devices()[:NCORES]
    mesh = Mesh(_np.asarray(devices), ("core",))
    sharded = jax.jit(
        shard_map(_body, mesh=mesh,
                  in_specs=(PartitionSpec("core"),) * (n_params + n_outs),
                  out_specs=(PartitionSpec("core"),) * n_outs,
                  check_rep=False),
        donate_argnums=donate, keep_unused=True,
    )

    def run(in_maps):
        concat_in = [
            np.concatenate([np.asarray(in_maps[c][nm]) for c in range(NCORES)], axis=0)
            for nm in in_names[:n_params]
        ]
        concat_zeros = [np.zeros((NCORES * z.shape[0], *z.shape[1:]), z.dtype)
                        for z in zero_outs]
        out_arrs = sharded(*concat_in, *concat_zeros)
        return [
            {nm: np.asarray(out_arrs[i]).reshape(NCORES, *out_avals[i].shape)[c]
             for i, nm in enumerate(out_names)}
            for c in range(NCORES)
        ]

    run.sharded = sharded
    run.zero_outs = zero_outs
    run.in_names = in_names[:n_params]
    run.out_names = out_names
    run.out_avals = out_avals
    return run


_BUILT = {}


def kernel(**inputs):
    a = np.asarray(inputs["a"], np.float32)
    s_prev = np.asarray(inputs["s_prev"], np.float32)
    W1 = np.asarray(inputs["W1"], np.float32)
    b1 = np.asarray(inputs["b1"], np.float32)
    W2 = np.asarray(inputs["W2"], np.float32)
    b2 = np.asarray(inputs["b2"], np.float32)
    w_c = np.asarray(inputs["w_c"], np.float32)
    w_u = np.asarray(inputs["w_u"], np.float32)
    w_f = np.asarray(inputs["w_f"], np.float32)
    w_o = np.asarray(inputs["w_o"], np.float32)
    b_c = np.asarray(inputs["b_c"], np.float32)
    b_u = np.asarray(inputs["b_u"], np.float32)
    b_f = np.asarray(inputs["b_f"], np.float32)
    b_o = np.asarray(inputs["b_o"], np.float32)
    wo = int(np.asarray(inputs["word_output"]))

    if wo not in _BUILT:
        nc_ = _build(wo)
        _BUILT[wo] = (nc_, _make_runner(nc_))
    nc, runner = _BUILT[wo]

    W1aT = np.zeros((F, MIDP), np.float16)
    W1aT[:, :MID] = W1[:, :F].T
    W1sT = np.zeros((O, MIDP), np.float16)
    W1sT[:, :MID] = W1[:, F:].T
    W2p = np.zeros((MIDP,), np.float32)
    W2p[:MID] = W2[0]
    W2c = W2p.reshape(8, 128).T.astype(np.float16)
    b1p = np.zeros((MIDP,), np.float32)
    b1p[:MID] = b1
    b1T = b1p.reshape(8, 128).T.copy()
    # Wg col order [c, u, f, o]
    WgT = np.concatenate([w.T for w in (w_c, w_u, w_f, w_o)], axis=1).astype(np.float16)
    bg = np.concatenate([b_c, b_u, b_f, b_o]).reshape(1, 4 * O).astype(np.float16)
    common = {
        "W1aT": W1aT, "W1sT": W1sT, "W2c": W2c, "b1T": b1T,
        "b2c": np.full((128, 1), float(b2.reshape(-1)[0]), np.float32),
        "WgT": WgT,
        "bgT2": bg.reshape(16, 128).astype(np.float16),
        "ind16": np.repeat(np.eye(16, dtype=np.float16), 16, axis=1),
        "zrow": np.zeros((1, 64), np.float16),
        "onesrh": np.ones((1, 128), np.float16),
        "onesc": np.ones((128, 1), np.float16),
        "onesr": np.ones((1, 128), np.float32),
        "ones16": np.ones((1, BC), np.float16),
    }
    in_maps = []
    for c in range(NCORES):
        b0 = c * BC
        ac = a[b0:b0 + BC]
        # sPT[p, oc*16+b] = s_prev[b, oc*128+p]
        sPT = np.ascontiguousarray(
            s_prev[b0:b0 + BC].reshape(BC, 4, 128).transpose(2, 1, 0).reshape(128, 64)
        ).astype(np.float16)
        in_maps.append({
            **common,
            "aT": np.ascontiguousarray(ac.transpose(2, 0, 1).reshape(F, BT)).astype(np.float16),
            "aN": np.ascontiguousarray(ac.reshape(BT, F)).astype(np.float16),
            "sPT": sPT,
        })

    results = None
    for attempt in range(4):
        try:
            results = runner(in_maps)
            break
        except Exception:
            if attempt == 3:
                raise
            import time as _time
            _time.sleep(1.0)
            if attempt >= 1:
                runner = _make_runner(nc)
                _BUILT[wo] = (nc, runner)
    out = np.empty((B, wo, O), np.float32)
    for c in range(NCORES):
        # buf [wo, 128, 64]; out[t, b, oc*128+p] = buf[t, p, oc*16+b]
        buf = results[c]["out"].reshape(wo, 128, 4, BC)
        out[c * BC:(c + 1) * BC] = buf.transpose(3, 0, 2, 1).reshape(BC, wo, O)
    return out


# revision 3
# speedup vs baseline: 1.0032x; 1.0032x over previous
"""Attention-decoder (B=128, T=256, F=512, O=512, MID=1000, 32 steps) on 8 trn2 cores.

v3: everything transposed. All per-step matmuls produce [128, tiny] outputs
(out-free-size is what the PE pays for): logits come out as logitsT [t, b]
columns (lhsT = tanh tile, rhs = W2 column), context as ctxT [f, b] (lhsT =
aN chunk, rhs = e column), gates as gT [o, b] (lhsT = WgT chunk, rhs = xaT).
The LSTM state stays in [o-chunk, b] layout so no transposes exist anywhere.
Softmax is unnormalized exp (exp(relu(x)) == max(exp(x),1), logits < 5) with
the 1/sum folded into the e-scatter; sums via ones-column matmuls.
"""
import sys
import numpy as np

sys.path.insert(0, "/opt/trn_rl_repo")

B, T, F, O, MID = 128, 256, 512, 512, 1000
MIDP = 1024  # padded
NCORES = 8
BC = B // NCORES  # 16 batch per core
BT = BC * T       # 4096


def _build(wo: int):
    import concourse.bass as bass
    import concourse.bacc as bacc
    import concourse.mybir as mybir
    from concourse.tile import TileContext

    f16 = mybir.dt.float16
    f32 = mybir.dt.float32
    AF = mybir.ActivationFunctionType
    OP = mybir.AluOpType

    nc = bacc.Bacc()
    aT_d = nc.dram_tensor("aT", [F, BT], f16, kind="ExternalInput")
    aN_d = nc.dram_tensor("aN", [BT, F], f16, kind="ExternalInput")
    W1aT_d = nc.dram_tensor("W1aT", [F, MIDP], f16, kind="ExternalInput")
    W1sT_d = nc.dram_tensor("W1sT", [O, MIDP], f16, kind="ExternalInput")
    W2c_d = nc.dram_tensor("W2c", [128, 8], f16, kind="ExternalInput")
    b1T_d = nc.dram_tensor("b1T", [128, 8], f32, kind="ExternalInput")
    b2c_d = nc.dram_tensor("b2c", [128, 1], f32, kind="ExternalInput")
    WgT_d = nc.dram_tensor("WgT", [O + F, 4 * O], f16, kind="ExternalInput")
    bgT2_d = nc.dram_tensor("bgT2", [16, 128], f16, kind="ExternalInput")
    ind_d = nc.dram_tensor("ind16", [16, 256], f16, kind="ExternalInput")
    zrow_d = nc.dram_tensor("zrow", [1, 64], f16, kind="ExternalInput")
    onesrh_d = nc.dram_tensor("onesrh", [1, 128], f16, kind="ExternalInput")
    sPT_d = nc.dram_tensor("sPT", [128, 64], f16, kind="ExternalInput")
    onesc_d = nc.dram_tensor("onesc", [128, 1], f16, kind="ExternalInput")
    onesr_d = nc.dram_tensor("onesr", [1, 128], f32, kind="ExternalInput")
    ones_d = nc.dram_tensor("ones16", [1, BC], f16, kind="ExternalInput")
    out_d = nc.dram_tensor("out", [wo, 128, 64], f32, kind="ExternalOutput")

    with TileContext(nc) as tc:
        with (
            tc.tile_pool(name="const", bufs=1) as cp,
            tc.tile_pool(name="hq", bufs=2) as hqp,
            tc.tile_pool(name="th", bufs=2) as thp,
            tc.tile_pool(name="wp", bufs=2) as wp,
            tc.tile_pool(name="small", bufs=2) as sp,
            tc.tile_pool(name="astream", bufs=1) as app,
            tc.tile_pool(name="pstf", bufs=2, space="PSUM") as pst,
            tc.tile_pool(name="psbig", bufs=1, space="PSUM") as psbig,
            tc.tile_pool(name="pslt", bufs=2, space="PSUM") as plp,
            tc.tile_pool(name="psct", bufs=2, space="PSUM") as pcp,
            tc.tile_pool(name="psgt", bufs=1, space="PSUM") as pgp,
        ):
            dma = nc.sync.dma_start

            # ---- constant loads ----
            aN_sb = {}
            for b in range(BC):
                for tcn in range(2):
                    t_ = cp.tile([128, F], f16, tag=f"aN{b}_{tcn}", name=f"aN{b}_{tcn}")
                    dma(t_[:], aN_d[b * T + tcn * 128: b * T + (tcn + 1) * 128, :])
                    aN_sb[(b, tcn)] = t_
            W1aT_sb, W1sT_sb, WgT_sb = [], [], []
            for kc in range(4):
                t_ = cp.tile([128, MIDP], f16, tag=f"w1a{kc}", name=f"w1a{kc}")
                dma(t_[:], W1aT_d[kc * 128:(kc + 1) * 128, :])
                W1aT_sb.append(t_)
            for kc in range(4):
                t_ = cp.tile([128, MIDP], f16, tag=f"w1s{kc}", name=f"w1s{kc}")
                dma(t_[:], W1sT_d[kc * 128:(kc + 1) * 128, :])
                W1sT_sb.append(t_)
            for kc in range(8):
                t_ = cp.tile([128, 4 * O], f16, tag=f"wg{kc}", name=f"wg{kc}")
                dma(t_[:], WgT_d[kc * 128:(kc + 1) * 128, :])
                WgT_sb.append(t_)
            W2_sb = cp.tile([128, 8], f16, tag="w2", name="w2")
            dma(W2_sb[:], W2c_d[:])
            b1T_sb = cp.tile([128, 8], f32, tag="b1t", name="b1t")
            dma(b1T_sb[:], b1T_d[:])
            b2c_sb = cp.tile([128, 1], f32, tag="b2c", name="b2c")
            dma(b2c_sb[:], b2c_d[:])
            bgT2_sb = cp.tile([16, 128], f16, tag="bgT2", name="bgT2")
            dma(bgT2_sb[:], bgT2_d[:])
            ind_sb = cp.tile([16, 256], f16, tag="ind16", name="ind16")
            dma(ind_sb[:], ind_d[:])
            zrow_sb = cp.tile([1, 64], f16, tag="zrow", name="zrow")
            dma(zrow_sb[:], zrow_d[:])
            onesrh_sb = cp.tile([1, 128], f16, tag="onesrh", name="onesrh")
            dma(onesrh_sb[:], onesrh_d[:])
            onesc_sb = cp.tile([128, 1], f16, tag="onesc", name="onesc")
            dma(onesc_sb[:], onesc_d[:])
            onesr_sb = cp.tile([1, 128], f32, tag="onesr", name="onesr")
            dma(onesr_sb[:], onesr_d[:])
            ones_sb = cp.tile([1, BC], f16, tag="ones", name="ones")
            dma(ones_sb[:], ones_d[:])

            s16 = wp.tile([128, 64], f16, tag="s16", name="s16")
            dma(s16[:], sPT_d[:])
            cT_prev = wp.tile([128, 64], f32, tag="cT", name="cT")
            nc.vector.memset(cT_prev[:], 0.0)

            # ---- precompute pre = (a @ W1a.T).T : [MID_p, (b,t)] fp16 ----
            pre_sb = []
            for mc in range(8):
                pre_sb.append(cp.tile([128, BT], f16, tag=f"pre{mc}", name=f"pre{mc}"))
            for ns in range(8):
                a_sl = []
                for kc in range(4):
                    t_ = app.tile([128, 512], f16, tag=f"astr{kc}", name=f"astr{kc}")
                    dma(t_[:], aT_d[kc * 128:(kc + 1) * 128, ns * 512:(ns + 1) * 512])
                    a_sl.append(t_)
                for mc in range(8):
                    ps = psbig.tile([128, 512], f32, tag="psbig", name="psbig")
                    for kc in range(4):
                        nc.tensor.matmul(
                            ps[:],
                            W1aT_sb[kc][:, mc * 128:(mc + 1) * 128],
                            a_sl[kc][:],
                            start=(kc == 0), stop=(kc == 3),
                        )
                    dst = pre_sb[mc][:, ns * 512:(ns + 1) * 512]
                    if mc % 2 == 0:
                        nc.scalar.copy(dst, ps[:])
                    else:
                        nc.vector.tensor_copy(dst, ps[:])

            # ---- decode steps ----
            for t in range(wo):
                # u.T = W1s @ s.T + b1 : [MID_p(128x8), b] fp32
                uT = wp.tile([128, 128], f32, tag="uT", name="uT")
                s16_cur = s16

                def uprep(mc):
                    psu = pst.tile([128, BC], f32, tag="pstf", name="psu")
                    for kc in range(4):
                        nc.tensor.matmul(
                            psu[:],
                            W1sT_sb[kc][:, mc * 128:(mc + 1) * 128],
                            s16_cur[:, kc * 16:(kc + 1) * 16],
                            start=(kc == 0), stop=(kc == 3),
                        )
                    nc.vector.tensor_scalar(
                        out=uT[:, mc * BC:(mc + 1) * BC], in0=psu[:],
                        scalar1=b1T_sb[:, mc:mc + 1], scalar2=None, op0=OP.add,
                    )

                # gates gT [128o, (g,oc,b)]: bias + s-part early; ctx in tail
                gT = pgp.tile([128, 256], f32, tag="gT", name="gT")

                def gates_early():
                    nc.tensor.matmul(gT[:], bgT2_sb[:], ind_sb[:], start=True, stop=False)
                    for g in range(4):
                        for oc in range(4):
                            col = (g * 4 + oc) * BC
                            for fc in range(4):
                                nc.tensor.matmul(
                                    gT[:, col:col + BC],
                                    WgT_sb[fc][:, g * O + oc * 128: g * O + (oc + 1) * 128],
                                    s16_cur[:, fc * 16:(fc + 1) * 16],
                                    start=False, stop=False)

                # attention: logitsT columns [t, (tcn,b)] via lhsT=tanh chunks
                plqs = [None] * 4
                pcqs = [None] * 4
                psS = pst.tile([1, BC], f32, tag="pstf", name="psS")
                einv = sp.tile([1, BC], f32, tag="einv", name="einv")
                cxT = sp.tile([128, 64], f16, tag="cxT", name="cxT", bufs=1)

                def attn_tail(q):
                    # exp of quad q's logitsT columns (reads PSUM directly)
                    ET = sp.tile([128, 8], f16, tag="ET", name="ET")
                    nc.scalar.activation(ET[:], plqs[q][:], AF.Exp,
                                         bias=b2c_sb[:, 0:1], scale=1.0)
                    # e = max(exp, 1)  (== exp(relu(logit)))
                    st2 = sp.tile([128, 8], f16, tag="st2", name="st2")
                    nc.vector.tensor_scalar(out=st2[:], in0=ET[:], scalar1=1.0,
                                            scalar2=None, op0=OP.max)
                    # unnormalized ctxT columns (lhsT = aN chunk, rhs = e col),
                    # overlapped with the 1/sum computation
                    pcq = pcp.tile([128, BC], f32, tag="pcq", name="pcq")
                    pcqs[q] = pcq
                    nc.tensor.matmul(pcq[:], onesrh_sb[:], zrow_sb[0:1, 0:BC],
                                     start=True, stop=False)
                    for bl in range(4):
                        b = q * 4 + bl
                        for fc in range(4):
                            for tcn in range(2):
                                nc.tensor.matmul(
                                    pcq[:, fc * 4 + bl: fc * 4 + bl + 1],
                                    aN_sb[(b, tcn)][:, fc * 128:(fc + 1) * 128],
                                    st2[:, tcn * 4 + bl: tcn * 4 + bl + 1],
                                    start=False,
                                    stop=(bl == 3 and fc == 3 and tcn == 1))
                    # per-batch sums via ones-column matmul (accum over tcn)
                    for tcn in range(2):
                        nc.tensor.matmul(psS[0:1, q * 4:(q + 1) * 4], onesc_sb[:],
                                         st2[:, tcn * 4:(tcn + 1) * 4],
                                         start=(tcn == 0), stop=(tcn == 1))
                    nc.vector.reciprocal(einv[0:1, q * 4:(q + 1) * 4],
                                         psS[0:1, q * 4:(q + 1) * 4])
                    pin = pst.tile([128, BC], f32, tag="pstf", name="pin")
                    nc.tensor.matmul(pin[:, 0:4], onesr_sb[:],
                                     einv[0:1, q * 4:(q + 1) * 4],
                                     start=True, stop=True)
                    # normalize into SBUF f16 ctxT for this quad's columns
                    for fc in range(4):
                        nc.vector.tensor_tensor(
                            out=cxT[:, fc * BC + q * 4: fc * BC + (q + 1) * 4],
                            in0=pcq[:, fc * 4:(fc + 1) * 4],
                            in1=pin[:, 0:4], op=OP.mult)
                    for g in range(4):
                        for oc in range(4):
                            col = (g * 4 + oc) * BC + q * 4
                            for fc in range(4):
                                nc.tensor.matmul(
                                    gT[:, col:col + 4],
                                    WgT_sb[4 + fc][:, g * O + oc * 128: g * O + (oc + 1) * 128],
                                    cxT[:, fc * BC + q * 4: fc * BC + (q + 1) * 4],
                                    start=False,
                                    stop=(q == 3 and g == 3 and oc == 3 and fc == 3))

                def quad_open(q):
                    plq = plp.tile([128, 8], f32, tag="plq", name="plq")
                    plqs[q] = plq
                    nc.tensor.matmul(plq[:], onesrh_sb[:], zrow_sb[0:1, 0:8],
                                     start=True, stop=False)

                def block_pre(q, mcp, split=1):
                    hq_t = hqp.tile([128, 4096], f16, tag="hq", name="hq")
                    th_t = thp.tile([128, 4096], f16, tag="th", name="th")
                    nsub = 4 // split
                    for sub in range(split):
                        for mcl in range(sub * nsub, (sub + 1) * nsub):
                            mc = mcp * 4 + mcl
                            for bl in range(4):
                                b = q * 4 + bl
                                nc.vector.tensor_scalar(
                                    out=hq_t[:, mcl * 1024 + bl * 256: mcl * 1024 + (bl + 1) * 256],
                                    in0=pre_sb[mc][:, b * 256:(b + 1) * 256],
                                    scalar1=uT[:, mc * BC + b: mc * BC + b + 1],
                                    scalar2=None, op0=OP.add,
                                )
                        nc.scalar.activation(
                            th_t[:, sub * nsub * 1024:(sub + 1) * nsub * 1024],
                            hq_t[:, sub * nsub * 1024:(sub + 1) * nsub * 1024],
                            AF.Tanh)
                    return th_t

                def block_mm(q, mcp, th_t):
                    plq = plqs[q]
                    for mcl in range(4):
                        mc = mcp * 4 + mcl
                        for bl in range(4):
                            for tcn in range(2):
                                nc.tensor.matmul(
                                    plq[:, tcn * 4 + bl: tcn * 4 + bl + 1],
                                    th_t[:, mcl * 1024 + bl * 256 + tcn * 128:
                                         mcl * 1024 + bl * 256 + (tcn + 1) * 128],
                                    W2_sb[:, mc:mc + 1],
                                    start=False,
                                    stop=(mc == 7 and bl == 3 and tcn == 1),
                                )

                uprep(0)
                quad_open(0)
                hq00 = hqp.tile([128, 4096], f16, tag="hq", name="hq")
                th00 = thp.tile([128, 4096], f16, tag="th", name="th")

                def sub00(mcl):
                    for bl in range(4):
                        nc.vector.tensor_scalar(
                            out=hq00[:, mcl * 1024 + bl * 256: mcl * 1024 + (bl + 1) * 256],
                            in0=pre_sb[mcl][:, bl * 256:(bl + 1) * 256],
                            scalar1=uT[:, mcl * BC + bl: mcl * BC + bl + 1],
                            scalar2=None, op0=OP.add,
                        )
                    nc.scalar.activation(th00[:, mcl * 1024:(mcl + 1) * 1024],
                                         hq00[:, mcl * 1024:(mcl + 1) * 1024],
                                         AF.Tanh)

                uprep(1)
                sub00(0)
                uprep(2)
                uprep(3)
                sub00(1)
                for mc_ in range(4, 8):
                    uprep(mc_)
                sub00(2)
                gates_early()
                sub00(3)
                block_mm(0, 0, th00)
                block_mm(0, 1, block_pre(0, 1))
                for q in range(1, 4):
                    quad_open(q)
                    for mcp_ in range(2):
                        block_mm(q, mcp_, block_pre(q, mcp_))
                    attn_tail(q - 1)
                attn_tail(3)

                # activations in [128, (g,oc,b)] layout: g0=cand tanh, g1..3 sigmoid
                gact = sp.tile([128, 64], f32, tag="gact", name="gact", bufs=1)
                nc.scalar.activation(gact[:], gT[:, 0:64], AF.Tanh)
                gsg = sp.tile([128, 192], f32, tag="gsg", name="gsg", bufs=1)
                nc.scalar.activation(gsg[:], gT[:, 64:256], AF.Tanh, scale=0.5)
                gsig = sp.tile([128, 192], f32, tag="gsig", name="gsig", bufs=1)
                nc.vector.tensor_scalar(out=gsig[:], in0=gsg[:], scalar1=0.5,
                                        scalar2=0.5, op0=OP.mult, op1=OP.add)

                t1 = sp.tile([128, 64], f32, tag="t1", name="t1", bufs=1)
                nc.vector.tensor_tensor(out=t1[:], in0=gsig[:, 0:64], in1=gact[:], op=OP.mult)
                t2 = sp.tile([128, 64], f32, tag="t2", name="t2", bufs=1)
                nc.vector.tensor_tensor(out=t2[:], in0=gsig[:, 64:128], in1=cT_prev[:], op=OP.mult)
                cT_new = wp.tile([128, 64], f32, tag="cT", name="cT")
                nc.vector.tensor_tensor(out=cT_new[:], in0=t1[:], in1=t2[:], op=OP.add)
                tch = sp.tile([128, 64], f32, tag="tch", name="tch", bufs=1)
                nc.scalar.activation(tch[:], cT_new[:], AF.Tanh)
                if t + 1 < wo:
                    s16_new = wp.tile([128, 64], f16, tag="s16", name="s16")
                    nc.vector.tensor_tensor(out=s16_new[:], in0=gsig[:, 128:192],
                                            in1=tch[:], op=OP.mult)
                    s16 = s16_new
                sT32 = wp.tile([128, 64], f32, tag="sT32", name="sT32")
                nc.gpsimd.tensor_tensor(out=sT32[:], in0=gsig[:, 128:192], in1=tch[:],
                                        op=OP.mult)
                dma(out_d[t, :, :], sT32[:])
                cT_prev = cT_new
    nc.compile()
    return nc


def _make_runner(nc):
    """Build the sharded jit callable ONCE per module (run_bass_via_pjrt
    rebuilds it per call, costing seconds of retrace/recompile)."""
    import jax
    import numpy as _np
    from jax.sharding import Mesh, PartitionSpec
    from jax.experimental.shard_map import shard_map
    from concourse import bass2jax, mybir

    bass2jax.install_neuronx_cc_hook()
    partition_name = nc.partition_id_tensor.name if nc.partition_id_tensor else None
    in_names, out_names, out_avals, zero_outs = [], [], [], []
    for alloc in nc.m.functions[0].allocations:
        if not isinstance(alloc, mybir.MemoryLocationSet):
            continue
        name = alloc.memorylocations[0].name
        if alloc.kind == "ExternalInput":
            if name != partition_name:
                in_names.append(name)
        elif alloc.kind == "ExternalOutput":
            shape = tuple(alloc.tensor_shape)
            dtype = mybir.dt.np(alloc.dtype)
            out_names.append(name)
            out_avals.append(jax.core.ShapedArray(shape, dtype))
            zero_outs.append(_np.zeros(shape, dtype))
    n_params = len(in_names)
    n_outs = len(out_avals)
    in_names_all = list(in_names) + list(out_names)
    if partition_name is not None:
        in_names_all.append(partition_name)

    def _body(*args):
        operands = list(args)
        if partition_name is not None:
            operands.append(bass2jax.partition_id_tensor())
        outs = bass2jax._bass_exec_p.bind(
            *operands,
            out_avals=tuple(out_avals),
            in_names=tuple(in_names_all),
            out_names=tuple(out_names),
            lowering_input_output_aliases=(),
            sim_require_finite=True,
            sim_require_nnan=True,
            nc=nc,
        )
        return tuple(outs)

    donate = tuple(range(n_params, n_params + n_outs))
    devices = jax.devices()[:NCORES]
    mesh = Mesh(_np.asarray(devices), ("core",))
    sharded = jax.jit(
        shard_map(_body, mesh=mesh,
                  in_specs=(PartitionSpec("core"),) * (n_params + n_outs),
                  out_specs=(PartitionSpec("core"),) * n_outs,
                  check_rep=False),
        donate_argnums=donate, keep_unused=True,
    )

    def run(in_maps):
        concat_in = [
            np.concatenate([np.asarray(in_maps[c][nm]) for c in range(NCORES)], axis=0)
            for nm in in_names[:n_params]
        ]
        concat_zeros = [np.zeros((NCORES * z.shape[0], *z.shape[1:]), z.dtype)
                        for z in zero_outs]
        out_arrs = sharded(*concat_in, *concat_zeros)
        return [
            {nm: np.asarray(out_arrs[i]).reshape(NCORES, *out_avals[i].shape)[c]
             for i, nm in enumerate(out_names)}
            for c in range(NCORES)
        ]

    run.sharded = sharded
    run.zero_outs = zero_outs
    run.in_names = in_names[:n_params]
    run.out_names = out_names
    run.out_avals = out_avals
    return run


_BUILT = {}


def kernel(**inputs):
    a = np.asarray(inputs["a"], np.float32)
    s_prev = np.asarray(inputs["s_prev"], np.float32)
    W1 = np.asarray(inputs["W1"], np.float32)
    b1 = np.asarray(inputs["b1"], np.float32)
    W2 = np.asarray(inputs["W2"], np.float32)
    b2 = np.asarray(inputs["b2"], np.float32)
    w_c = np.asarray(inputs["w_c"], np.float32)
    w_u = np.asarray(inputs["w_u"], np.float32)
    w_f = np.asarray(inputs["w_f"], np.float32)
    w_o = np.asarray(inputs["w_o"], np.float32)
    b_c = np.asarray(inputs["b_c"], np.float32)
    b_u = np.asarray(inputs["b_u"], np.float32)
    b_f = np.asarray(inputs["b_f"], np.float32)
    b_o = np.asarray(inputs["b_o"], np.float32)
    wo = int(np.asarray(inputs["word_output"]))

    if wo not in _BUILT:
        nc_ = _build(wo)
        _BUILT[wo] = (nc_, _make_runner(nc_))
    nc, runner = _BUILT[wo]

    W1aT = np.zeros((F, MIDP), np.float16)
    W1aT[:, :MID] = W1[:, :F].T
    W1sT = np.zeros((O, MIDP), np.float16)
    W1sT[:, :MID] = W1[:, F:].T
    W2p = np.zeros((MIDP,), np.float32)
    W2p[:MID] = W2[0]
    W2c = W2p.reshape(8, 128).T.astype(np.float16)
    b1p = np.zeros((MIDP,), np.float32)
    b1p[:MID] = b1
    b1T = b1p.reshape(8, 128).T.copy()
    # Wg col order [c, u, f, o]
    WgT = np.concatenate([w.T for w in (w_c, w_u, w_f, w_o)], axis=1).astype(np.float16)
    bg = np.concatenate([b_c, b_u, b_f, b_o]).reshape(1, 4 * O).astype(np.float16)
    common = {
        "W1aT": W1aT, "W1sT": W1sT, "W2c": W2c, "b1T": b1T,
        "b2c": np.full((128, 1), float(b2.reshape(-1)[0]), np.float32),
        "WgT": WgT,
        "bgT2": bg.reshape(16, 128).astype(np.float16),
        "ind16": np.repeat(np.eye(16, dtype=np.float16), 16, axis=1),
        "zrow": np.zeros((1, 64), np.float16),
        "onesrh": np.ones((1, 128), np.float16),
        "onesc": np.ones((128, 1), np.float16),
        "onesr": np.ones((1, 128), np.float32),
        "ones16": np.ones((1, BC), np.float16),
    }
    in_maps = []
    for c in range(NCORES):
        b0 = c * BC
        ac = a[b0:b0 + BC]
        # sPT[p, oc*16+b] = s_prev[b, oc*128+p]
        sPT = np.ascontiguousarray(
            s_prev[b0:b0 + BC].reshape(BC, 4, 128).transpose(2, 1, 0).reshape(128, 64)
        ).astype(np.float16)
        in_maps.append({
            **common,
            "aT": np.ascontiguousarray(ac.transpose(2, 0, 1).reshape(F, BT)).astype(np.float16),
            "aN": np.ascontiguousarray(ac.reshape(BT, F)).astype(np.float16),
            "sPT": sPT,
        })

    results = None
    for attempt in range(4):
        try:
            results = runner(in_maps)
            break
        except Exception:
            if attempt == 3:
                raise
            import time as _time
            _time.sleep(1.0)
            if attempt >= 1:
                runner = _make_runner(nc)
                _BUILT[wo] = (nc, runner)
    out = np.empty((B, wo, O), np.float32)
    for c in range(NCORES):
        # buf [wo, 128, 64]; out[t, b, oc*128+p] = buf[t, p, oc*16+b]
        buf = results[c]["out"].reshape(wo, 128, 4, BC)
        out[c * BC:(c + 1) * BC] = buf.transpose(3, 0, 2, 1).reshape(BC, wo, O)
    return out


# revision 4
# speedup vs baseline: 1.0074x; 1.0041x over previous
"""Attention-decoder (B=128, T=256, F=512, O=512, MID=1000, 32 steps) on 8 trn2 cores.

v3: everything transposed. All per-step matmuls produce [128, tiny] outputs
(out-free-size is what the PE pays for): logits come out as logitsT [t, b]
columns (lhsT = tanh tile, rhs = W2 column), context as ctxT [f, b] (lhsT =
aN chunk, rhs = e column), gates as gT [o, b] (lhsT = WgT chunk, rhs = xaT).
The LSTM state stays in [o-chunk, b] layout so no transposes exist anywhere.
Softmax is unnormalized exp (exp(relu(x)) == max(exp(x),1), logits < 5) with
the 1/sum folded into the e-scatter; sums via ones-column matmuls.
"""
import sys
import numpy as np

sys.path.insert(0, "/opt/trn_rl_repo")

B, T, F, O, MID = 128, 256, 512, 512, 1000
MIDP = 1024  # padded
NCORES = 8
BC = B // NCORES  # 16 batch per core
BT = BC * T       # 4096


def _build(wo: int):
    import concourse.bass as bass
    import concourse.bacc as bacc
    import concourse.mybir as mybir
    from concourse.tile import TileContext

    f16 = mybir.dt.float16
    f32 = mybir.dt.float32
    AF = mybir.ActivationFunctionType
    OP = mybir.AluOpType

    nc = bacc.Bacc()
    aT_d = nc.dram_tensor("aT", [F, BT], f16, kind="ExternalInput")
    aN_d = nc.dram_tensor("aN", [BT, F], f16, kind="ExternalInput")
    W1aT_d = nc.dram_tensor("W1aT", [F, MIDP], f16, kind="ExternalInput")
    W1sT_d = nc.dram_tensor("W1sT", [O, MIDP], f16, kind="ExternalInput")
    W2c_d = nc.dram_tensor("W2c", [128, 8], f16, kind="ExternalInput")
    b1T_d = nc.dram_tensor("b1T", [128, 8], f32, kind="ExternalInput")
    b2c_d = nc.dram_tensor("b2c", [128, 1], f32, kind="ExternalInput")
    WgT_d = nc.dram_tensor("WgT", [O + F, 4 * O], f16, kind="ExternalInput")
    bgT2_d = nc.dram_tensor("bgT2", [16, 128], f16, kind="ExternalInput")
    ind_d = nc.dram_tensor("ind16", [16, 256], f16, kind="ExternalInput")
    zrow_d = nc.dram_tensor("zrow", [1, 64], f16, kind="ExternalInput")
    onesrh_d = nc.dram_tensor("onesrh", [1, 128], f16, kind="ExternalInput")
    sPT_d = nc.dram_tensor("sPT", [128, 64], f16, kind="ExternalInput")
    onesc_d = nc.dram_tensor("onesc", [128, 1], f16, kind="ExternalInput")
    onesr_d = nc.dram_tensor("onesr", [1, 128], f32, kind="ExternalInput")
    ones_d = nc.dram_tensor("ones16", [1, BC], f16, kind="ExternalInput")
    out_d = nc.dram_tensor("out", [wo, 128, 64], f32, kind="ExternalOutput")

    with TileContext(nc) as tc:
        with (
            tc.tile_pool(name="const", bufs=1) as cp,
            tc.tile_pool(name="hq", bufs=2) as hqp,
            tc.tile_pool(name="th", bufs=2) as thp,
            tc.tile_pool(name="wp", bufs=2) as wp,
            tc.tile_pool(name="small", bufs=2) as sp,
            tc.tile_pool(name="astream", bufs=1) as app,
            tc.tile_pool(name="pstf", bufs=2, space="PSUM") as pst,
            tc.tile_pool(name="psbig", bufs=1, space="PSUM") as psbig,
            tc.tile_pool(name="pslt", bufs=2, space="PSUM") as plp,
            tc.tile_pool(name="psct", bufs=2, space="PSUM") as pcp,
            tc.tile_pool(name="psgt", bufs=1, space="PSUM") as pgp,
        ):
            dma = nc.sync.dma_start

            # ---- constant loads ----
            aN_sb = {}
            for b in range(BC):
                for tcn in range(2):
                    t_ = cp.tile([128, F], f16, tag=f"aN{b}_{tcn}", name=f"aN{b}_{tcn}")
                    dma(t_[:], aN_d[b * T + tcn * 128: b * T + (tcn + 1) * 128, :])
                    aN_sb[(b, tcn)] = t_
            W1aT_sb, W1sT_sb, WgT_sb = [], [], []
            for kc in range(4):
                t_ = cp.tile([128, MIDP], f16, tag=f"w1a{kc}", name=f"w1a{kc}")
                dma(t_[:], W1aT_d[kc * 128:(kc + 1) * 128, :])
                W1aT_sb.append(t_)
            for kc in range(4):
                t_ = cp.tile([128, MIDP], f16, tag=f"w1s{kc}", name=f"w1s{kc}")
                dma(t_[:], W1sT_d[kc * 128:(kc + 1) * 128, :])
                W1sT_sb.append(t_)
            for kc in range(8):
                t_ = cp.tile([128, 4 * O], f16, tag=f"wg{kc}", name=f"wg{kc}")
                dma(t_[:], WgT_d[kc * 128:(kc + 1) * 128, :])
                WgT_sb.append(t_)
            W2_sb = cp.tile([128, 8], f16, tag="w2", name="w2")
            dma(W2_sb[:], W2c_d[:])
            b1T_sb = cp.tile([128, 8], f32, tag="b1t", name="b1t")
            dma(b1T_sb[:], b1T_d[:])
            b2c_sb = cp.tile([128, 1], f32, tag="b2c", name="b2c")
            dma(b2c_sb[:], b2c_d[:])
            bgT2_sb = cp.tile([16, 128], f16, tag="bgT2", name="bgT2")
            dma(bgT2_sb[:], bgT2_d[:])
            ind_sb = cp.tile([16, 256], f16, tag="ind16", name="ind16")
            dma(ind_sb[:], ind_d[:])
            zrow_sb = cp.tile([1, 64], f16, tag="zrow", name="zrow")
            dma(zrow_sb[:], zrow_d[:])
            onesrh_sb = cp.tile([1, 128], f16, tag="onesrh", name="onesrh")
            dma(onesrh_sb[:], onesrh_d[:])
            onesc_sb = cp.tile([128, 1], f16, tag="onesc", name="onesc")
            dma(onesc_sb[:], onesc_d[:])
            onesr_sb = cp.tile([1, 128], f32, tag="onesr", name="onesr")
            dma(onesr_sb[:], onesr_d[:])
            ones_sb = cp.tile([1, BC], f16, tag="ones", name="ones")
            dma(ones_sb[:], ones_d[:])

            s16 = wp.tile([128, 64], f16, tag="s16", name="s16")
            dma(s16[:], sPT_d[:])
            cT_prev = wp.tile([128, 64], f32, tag="cT", name="cT")
            nc.vector.memset(cT_prev[:], 0.0)

            # ---- precompute pre = (a @ W1a.T).T : [MID_p, (b,t)] fp16 ----
            pre_sb = []
            for mc in range(8):
                pre_sb.append(cp.tile([128, BT], f16, tag=f"pre{mc}", name=f"pre{mc}"))
            for ns in range(8):
                a_sl = []
                for kc in range(4):
                    t_ = app.tile([128, 512], f16, tag=f"astr{kc}", name=f"astr{kc}")
                    dma(t_[:], aT_d[kc * 128:(kc + 1) * 128, ns * 512:(ns + 1) * 512])
                    a_sl.append(t_)
                for mc in range(8):
                    ps = psbig.tile([128, 512], f32, tag="psbig", name="psbig")
                    for kc in range(4):
                        nc.tensor.matmul(
                            ps[:],
                            W1aT_sb[kc][:, mc * 128:(mc + 1) * 128],
                            a_sl[kc][:],
                            start=(kc == 0), stop=(kc == 3),
                        )
                    dst = pre_sb[mc][:, ns * 512:(ns + 1) * 512]
                    if mc % 2 == 0:
                        nc.scalar.copy(dst, ps[:])
                    else:
                        nc.vector.tensor_copy(dst, ps[:])

            # ---- decode steps ----
            for t in range(wo):
                # u.T = W1s @ s.T + b1 : [MID_p(128x8), b] fp32
                uT = wp.tile([128, 128], f32, tag="uT", name="uT")
                s16_cur = s16

                def uprep(mc):
                    psu = pst.tile([128, BC], f32, tag="pstf", name="psu")
                    for kc in range(4):
                        nc.tensor.matmul(
                            psu[:],
                            W1sT_sb[kc][:, mc * 128:(mc + 1) * 128],
                            s16_cur[:, kc * 16:(kc + 1) * 16],
                            start=(kc == 0), stop=(kc == 3),
                        )
                    nc.vector.tensor_scalar(
                        out=uT[:, mc * BC:(mc + 1) * BC], in0=psu[:],
                        scalar1=b1T_sb[:, mc:mc + 1], scalar2=None, op0=OP.add,
                    )

                # gates gT [128o, (g,oc,b)]: bias + s-part early; ctx in tail
                gT = pgp.tile([128, 256], f32, tag="gT", name="gT")

                def gates_early():
                    nc.tensor.matmul(gT[:], bgT2_sb[:], ind_sb[:], start=True, stop=False)
                    for g in range(4):
                        for oc in range(4):
                            col = (g * 4 + oc) * BC
                            for fc in range(4):
                                nc.tensor.matmul(
                                    gT[:, col:col + BC],
                                    WgT_sb[fc][:, g * O + oc * 128: g * O + (oc + 1) * 128],
                                    s16_cur[:, fc * 16:(fc + 1) * 16],
                                    start=False, stop=False)

                # attention: logitsT columns [t, (tcn,b)] via lhsT=tanh chunks
                plqs = [None] * 4
                pcqs = [None] * 4
                psS = pst.tile([1, BC], f32, tag="pstf", name="psS")
                einv = sp.tile([1, BC], f32, tag="einv", name="einv")
                cxT = sp.tile([128, 64], f16, tag="cxT", name="cxT", bufs=1)

                def attn_tail(q):
                    # exp of quad q's logitsT columns (reads PSUM directly)
                    ET = sp.tile([128, 8], f16, tag="ET", name="ET")
                    nc.scalar.activation(ET[:], plqs[q][:], AF.Exp,
                                         bias=b2c_sb[:, 0:1], scale=1.0)
                    # e = max(exp, 1)  (== exp(relu(logit)))
                    st2 = sp.tile([128, 8], f16, tag="st2", name="st2")
                    nc.vector.tensor_scalar(out=st2[:], in0=ET[:], scalar1=1.0,
                                            scalar2=None, op0=OP.max)
                    # unnormalized ctxT columns (lhsT = aN chunk, rhs = e col),
                    # overlapped with the 1/sum computation
                    pcq = pcp.tile([128, BC], f32, tag="pcq", name="pcq")
                    pcqs[q] = pcq
                    nc.tensor.matmul(pcq[:], onesrh_sb[:], zrow_sb[0:1, 0:BC],
                                     start=True, stop=False)
                    for bl in range(4):
                        b = q * 4 + bl
                        for fc in range(4):
                            for tcn in range(2):
                                nc.tensor.matmul(
                                    pcq[:, fc * 4 + bl: fc * 4 + bl + 1],
                                    aN_sb[(b, tcn)][:, fc * 128:(fc + 1) * 128],
                                    st2[:, tcn * 4 + bl: tcn * 4 + bl + 1],
                                    start=False,
                                    stop=(bl == 3 and fc == 3 and tcn == 1))
                    # per-batch sums via ones-column matmul (accum over tcn)
                    for tcn in range(2):
                        nc.tensor.matmul(psS[0:1, q * 4:(q + 1) * 4], onesc_sb[:],
                                         st2[:, tcn * 4:(tcn + 1) * 4],
                                         start=(tcn == 0), stop=(tcn == 1))
                    nc.vector.reciprocal(einv[0:1, q * 4:(q + 1) * 4],
                                         psS[0:1, q * 4:(q + 1) * 4])
                    pin = pst.tile([128, BC], f32, tag="pstf", name="pin")
                    nc.tensor.matmul(pin[:, 0:4], onesr_sb[:],
                                     einv[0:1, q * 4:(q + 1) * 4],
                                     start=True, stop=True)
                    # normalize into SBUF f16 ctxT for this quad's columns
                    for fc in range(4):
                        nc.vector.tensor_tensor(
                            out=cxT[:, fc * BC + q * 4: fc * BC + (q + 1) * 4],
                            in0=pcq[:, fc * 4:(fc + 1) * 4],
                            in1=pin[:, 0:4], op=OP.mult)
                    for g in range(4):
                        for oc in range(4):
                            col = (g * 4 + oc) * BC + q * 4
                            for fc in range(4):
                                nc.tensor.matmul(
                                    gT[:, col:col + 4],
                                    WgT_sb[4 + fc][:, g * O + oc * 128: g * O + (oc + 1) * 128],
                                    cxT[:, fc * BC + q * 4: fc * BC + (q + 1) * 4],
                                    start=False,
                                    stop=(q == 3 and g == 3 and oc == 3 and fc == 3))

                def quad_open(q):
                    plq = plp.tile([128, 8], f32, tag="plq", name="plq")
                    plqs[q] = plq
                    nc.tensor.matmul(plq[:], onesrh_sb[:], zrow_sb[0:1, 0:8],
                                     start=True, stop=False)

                def block_pre(q, mcp, split=1):
                    hq_t = hqp.tile([128, 4096], f16, tag="hq", name="hq")
                    th_t = thp.tile([128, 4096], f16, tag="th", name="th")
                    nsub = 4 // split
                    for sub in range(split):
                        for mcl in range(sub * nsub, (sub + 1) * nsub):
                            mc = mcp * 4 + mcl
                            for bl in range(4):
                                b = q * 4 + bl
                                nc.vector.tensor_scalar(
                                    out=hq_t[:, mcl * 1024 + bl * 256: mcl * 1024 + (bl + 1) * 256],
                                    in0=pre_sb[mc][:, b * 256:(b + 1) * 256],
                                    scalar1=uT[:, mc * BC + b: mc * BC + b + 1],
                                    scalar2=None, op0=OP.add,
                                )
                        nc.scalar.activation(
                            th_t[:, sub * nsub * 1024:(sub + 1) * nsub * 1024],
                            hq_t[:, sub * nsub * 1024:(sub + 1) * nsub * 1024],
                            AF.Tanh)
                    return th_t

                def block_mm(q, mcp, th_t):
                    plq = plqs[q]
                    for mcl in range(4):
                        mc = mcp * 4 + mcl
                        for bl in range(4):
                            for tcn in range(2):
                                nc.tensor.matmul(
                                    plq[:, tcn * 4 + bl: tcn * 4 + bl + 1],
                                    th_t[:, mcl * 1024 + bl * 256 + tcn * 128:
                                         mcl * 1024 + bl * 256 + (tcn + 1) * 128],
                                    W2_sb[:, mc:mc + 1],
                                    start=False,
                                    stop=(mc == 7 and bl == 3 and tcn == 1),
                                )

                uprep(0)
                quad_open(0)
                hq00 = hqp.tile([128, 4096], f16, tag="hq", name="hq")
                th00 = thp.tile([128, 4096], f16, tag="th", name="th")

                def sub00(mcl):
                    for bl in range(4):
                        nc.vector.tensor_scalar(
                            out=hq00[:, mcl * 1024 + bl * 256: mcl * 1024 + (bl + 1) * 256],
                            in0=pre_sb[mcl][:, bl * 256:(bl + 1) * 256],
                            scalar1=uT[:, mcl * BC + bl: mcl * BC + bl + 1],
                            scalar2=None, op0=OP.add,
                        )
                    nc.scalar.activation(th00[:, mcl * 1024:(mcl + 1) * 1024],
                                         hq00[:, mcl * 1024:(mcl + 1) * 1024],
                                         AF.Tanh)

                uprep(1)
                sub00(0)
                uprep(2)
                uprep(3)
                sub00(1)
                for mc_ in range(4, 8):
                    uprep(mc_)
                sub00(2)
                gates_early()
                sub00(3)
                block_mm(0, 0, th00)
                block_mm(0, 1, block_pre(0, 1))
                for q in range(1, 4):
                    quad_open(q)
                    for mcp_ in range(2):
                        block_mm(q, mcp_, block_pre(q, mcp_))
                    attn_tail(q - 1)
                attn_tail(3)

                # activations in [128, (g,oc,b)] layout: g0=cand tanh, g1..3 sigmoid
                gact = sp.tile([128, 64], f32, tag="gact", name="gact", bufs=1)
                nc.scalar.activation(gact[:], gT[:, 0:64], AF.Tanh)
                gsg = sp.tile([128, 192], f32, tag="gsg", name="gsg", bufs=1)
                nc.scalar.activation(gsg[:], gT[:, 64:256], AF.Tanh, scale=0.5)
                gsig = sp.tile([128, 192], f32, tag="gsig", name="gsig", bufs=1)
                nc.vector.tensor_scalar(out=gsig[:], in0=gsg[:], scalar1=0.5,
                                        scalar2=0.5, op0=OP.mult, op1=OP.add)

                t1 = sp.tile([128, 64], f32, tag="t1", name="t1", bufs=1)
                nc.vector.tensor_tensor(out=t1[:], in0=gsig[:, 0:64], in1=gact[:], op=OP.mult)
                t2 = sp.tile([128, 64], f32, tag="t2", name="t2", bufs=1)
                nc.vector.tensor_tensor(out=t2[:], in0=gsig[:, 64:128], in1=cT_prev[:], op=OP.mult)
                cT_new = wp.tile([128, 64], f32, tag="cT", name="cT")
                nc.vector.tensor_tensor(out=cT_new[:], in0=t1[:], in1=t2[:], op=OP.add)
                tch = sp.tile([128, 64], f32, tag="tch", name="tch", bufs=1)
                nc.scalar.activation(tch[:], cT_new[:], AF.Tanh)
                if t + 1 < wo:
                    s16_new = wp.tile([128, 64], f16, tag="s16", name="s16")
                    nc.vector.tensor_tensor(out=s16_new[:], in0=gsig[:, 128:192],
                                            in1=tch[:], op=OP.mult)
                    s16 = s16_new
                sT32 = wp.tile([128, 64], f32, tag="sT32", name="sT32")
                nc.vector.tensor_tensor(out=sT32[:], in0=gsig[:, 128:192], in1=tch[:],
                                        op=OP.mult)
                dma(out_d[t, :, :], sT32[:])
                cT_prev = cT_new
    nc.compile()
    return nc


def _make_runner(nc):
    """Build the sharded jit callable ONCE per module (run_bass_via_pjrt
    rebuilds it per call, costing seconds of retrace/recompile)."""
    import jax
    import numpy as _np
    from jax.sharding import Mesh, PartitionSpec
    from jax.experimental.shard_map import shard_map
    from concourse import bass2jax, mybir

    bass2jax.install_neuronx_cc_hook()
    partition_name = nc.partition_id_tensor.name if nc.partition_id_tensor else None
    in_names, out_names, out_avals, zero_outs = [], [], [], []
    for alloc in nc.m.functions[0].allocations:
        if not isinstance(alloc, mybir.MemoryLocationSet):
            continue
        name = alloc.memorylocations[0].name
        if alloc.kind == "ExternalInput":
            if name != partition_name:
                in_names.append(name)
        elif alloc.kind == "ExternalOutput":
            shape = tuple(alloc.tensor_shape)
            dtype = mybir.dt.np(alloc.dtype)
            out_names.append(name)
            out_avals.append(jax.core.ShapedArray(shape, dtype))
            zero_outs.append(_np.zeros(shape, dtype))
    n_params = len(in_names)
    n_outs = len(out_avals)
    in_names_all = list(in_names) + list(out_names)
    if partition_name is not None:
        in_names_all.append(partition_name)

    def _body(*args):
        operands = list(args)
        if partition_name is not None:
            operands.append(bass2jax.partition_id_tensor())
        outs = bass2jax._bass_exec_p.bind(
            *operands,
            out_avals=tuple(out_avals),
            in_names=tuple(in_names_all),
            out_names=tuple(out_names),
            lowering_input_output_aliases=(),
            sim_require_finite=True,
            sim_require_nnan=True,
            nc=nc,
        )
        return tuple(outs)

    donate = tuple(range(n_params, n_params + n_outs))
    devices = jax.devices()[:NCORES]
    mesh = Mesh(_np.asarray(devices), ("core",))
    sharded = jax.jit(
        shard_map(_body, mesh=mesh,
                  in_specs=(PartitionSpec("core"),) * (n_params + n_outs),
                  out_specs=(PartitionSpec("core"),) * n_outs,
                  check_rep=False),
        donate_argnums=donate, keep_unused=True,
    )

    def run(in_maps):
        concat_in = [
            np.concatenate([np.asarray(in_maps[c][nm]) for c in range(NCORES)], axis=0)
            for nm in in_names[:n_params]
        ]
        concat_zeros = [np.zeros((NCORES * z.shape[0], *z.shape[1:]), z.dtype)
                        for z in zero_outs]
        out_arrs = sharded(*concat_in, *concat_zeros)
        return [
            {nm: np.asarray(out_arrs[i]).reshape(NCORES, *out_avals[i].shape)[c]
             for i, nm in enumerate(out_names)}
            for c in range(NCORES)
        ]

    run.sharded = sharded
    run.zero_outs = zero_outs
    run.in_names = in_names[:n_params]
    run.out_names = out_names
    run.out_avals = out_avals
    return run


_BUILT = {}


def kernel(**inputs):
    a = np.asarray(inputs["a"], np.float32)
    s_prev = np.asarray(inputs["s_prev"], np.float32)
    W1 = np.asarray(inputs["W1"], np.float32)
    b1 = np.asarray(inputs["b1"], np.float32)
    W2 = np.asarray(inputs["W2"], np.float32)
    b2 = np.asarray(inputs["b2"], np.float32)
    w_c = np.asarray(inputs["w_c"], np.float32)
    w_u = np.asarray(inputs["w_u"], np.float32)
    w_f = np.asarray(inputs["w_f"], np.float32)
    w_o = np.asarray(inputs["w_o"], np.float32)
    b_c = np.asarray(inputs["b_c"], np.float32)
    b_u = np.asarray(inputs["b_u"], np.float32)
    b_f = np.asarray(inputs["b_f"], np.float32)
    b_o = np.asarray(inputs["b_o"], np.float32)
    wo = int(np.asarray(inputs["word_output"]))

    if wo not in _BUILT:
        nc_ = _build(wo)
        _BUILT[wo] = (nc_, _make_runner(nc_))
    nc, runner = _BUILT[wo]

    W1aT = np.zeros((F, MIDP), np.float16)
    W1aT[:, :MID] = W1[:, :F].T
    W1sT = np.zeros((O, MIDP), np.float16)
    W1sT[:, :MID] = W1[:, F:].T
    W2p = np.zeros((MIDP,), np.float32)
    W2p[:MID] = W2[0]
    W2c = W2p.reshape(8, 128).T.astype(np.float16)
    b1p = np.zeros((MIDP,), np.float32)
    b1p[:MID] = b1
    b1T = b1p.reshape(8, 128).T.copy()
    # Wg col order [c, u, f, o]
    WgT = np.concatenate([w.T for w in (w_c, w_u, w_f, w_o)], axis=1).astype(np.float16)
    bg = np.concatenate([b_c, b_u, b_f, b_o]).reshape(1, 4 * O).astype(np.float16)
    common = {
        "W1aT": W1aT, "W1sT": W1sT, "W2c": W2c, "b1T": b1T,
        "b2c": np.full((128, 1), float(b2.reshape(-1)[0]), np.float32),
        "WgT": WgT,
        "bgT2": bg.reshape(16, 128).astype(np.float16),
        "ind16": np.repeat(np.eye(16, dtype=np.float16), 16, axis=1),
        "zrow": np.zeros((1, 64), np.float16),
        "onesrh": np.ones((1, 128), np.float16),
        "onesc": np.ones((128, 1), np.float16),
        "onesr": np.ones((1, 128), np.float32),
        "ones16": np.ones((1, BC), np.float16),
    }
    in_maps = []
    for c in range(NCORES):
        b0 = c * BC
        ac = a[b0:b0 + BC]
        # sPT[p, oc*16+b] = s_prev[b, oc*128+p]
        sPT = np.ascontiguousarray(
            s_prev[b0:b0 + BC].reshape(BC, 4, 128).transpose(2, 1, 0).reshape(128, 64)
        ).astype(np.float16)
        in_maps.append({
            **common,
            "aT": np.ascontiguousarray(ac.transpose(2, 0, 1).reshape(F, BT)).astype(np.float16),
            "aN": np.ascontiguousarray(ac.reshape(BT, F)).astype(np.float16),
            "sPT": sPT,
        })

    results = None
    for attempt in range(4):
        try:
            results = runner(in_maps)
            break
        except Exception:
            if attempt == 3:
                raise
            import time as _time
            _time.sleep(1.0)
            if attempt >= 1:
                runner = _make_runner(nc)
                _BUILT[wo] = (nc, runner)
    out = np.empty((B, wo, O), np.float32)
    for c in range(NCORES):
        # buf [wo, 128, 64]; out[t, b, oc*128+p] = buf[t, p, oc*16+b]
        buf = results[c]["out"].reshape(wo, 128, 4, BC)
        out[c * BC:(c + 1) * BC] = buf.transpose(3, 0, 2, 1).reshape(BC, wo, O)
    return out
